# revision 12
# baseline (speedup 1.0000x reference)
# Multi-head attention (N=2, S=2048, E=2048, H=16, Dk=128) on 8 NeuronCores.
#
# Sharding: 2 batches x 16 heads = 32 (n,h) pairs -> core c owns batch c//4,
# heads (c%4)*4 .. +4. The reference reshapes (N,H,S,Dk)->(N,S,H*Dk) without
# a head transpose, so rows [h*128,(h+1)*128) of the pre-projection matrix X
# (and hence of the final output) depend on head h only: each core computes
# 512 disjoint output rows and the host concatenates. No collectives.
#
# Device math per core (all matmuls fp32r, transposed layouts):
#   qT_c = Wq_c @ query[n].T   (hd x S)   kT_c same     v_c = value[n] @ Wv_c.T (S x hd)
#   sT   = k_h^T-tiles @ qT_h  (t x s)    expT = exp(sT/sqrt(Dk))
#   outT = v_h-tiles.T @ expT  (d x s)    denom via ones-matmul over acc(expT)
#   out  = X_h @ Wo.T + bo  with X_h^T k-tiles = strided views of outT
import numpy as np

D_MODEL = 2048
NHEAD = 16
DK = 128
N_BATCH = 2
SEQ = 2048
N_CORES = 8
HEADS_PER_CORE = 4


class Cfg:
    def __init__(self, S=SEQ, E=D_MODEL, NH=HEADS_PER_CORE, CH=256):
        assert S % 128 == 0 and E % 128 == 0
        self.S = S          # sequence length
        self.E = E          # model dim (contraction for projections)
        self.NH = NH        # heads per core
        self.CH = CH        # s-chunk width for attention phase
        self.NK = E // 128  # contraction tiles for projections / O-proj
        self.NT = S // 128  # t tiles (attention contraction)
        self.HDc = NH * DK  # head dims per core
        self.RPH = (S * DK) // E  # output rows per head (=128 at full size)
        assert self.RPH == 128, "O-proj layout assumes 128 rows per head"
        self.NCH = S // CH  # number of s-chunks
        assert S % CH == 0 and CH >= 256  # fp32r full-rate needs N>=256
        self.PCH = 512      # projection / O-proj free-dim chunk
        self.NPC = S // self.PCH   # projection s-chunks
        self.NOC = E // self.PCH   # O-proj output chunks


def build_program(cfg: Cfg):
    import concourse.bass as bass
    import concourse.tile as tile
    from concourse import bacc, mybir
    from contextlib import ExitStack

    fp32 = mybir.dt.float32
    fp32r = mybir.dt.float32r
    AF = mybir.ActivationFunctionType

    S, E, NH, CH = cfg.S, cfg.E, cfg.NH, cfg.CH
    NK, NT, HDc = cfg.NK, cfg.NT, cfg.HDc
    PCH, NPC, NOC, NCH = cfg.PCH, cfg.NPC, cfg.NOC, cfg.NCH
    inv_sqrt_dk = 1.0 / float(np.sqrt(DK))

    nc = bacc.Bacc("TRN2", target_bir_lowering=False, debug=False,
                   num_devices=N_CORES)

    # DRAM I/O (per-core values supplied via in_maps)
    xqT = nc.dram_tensor("xqT", [E, S], fp32r, kind="ExternalInput").ap()
    xkT = nc.dram_tensor("xkT", [E, S], fp32r, kind="ExternalInput").ap()
    xvT = nc.dram_tensor("xvT", [E, S], fp32r, kind="ExternalInput").ap()
    wqT = nc.dram_tensor("wqT", [E, HDc], fp32r, kind="ExternalInput").ap()
    wkT = nc.dram_tensor("wkT", [E, HDc], fp32r, kind="ExternalInput").ap()
    wvT = nc.dram_tensor("wvT", [E, HDc], fp32r, kind="ExternalInput").ap()
    woT = nc.dram_tensor("woT", [E, E], fp32r, kind="ExternalInput").ap()
    bq = nc.dram_tensor("bq", [128, NH], fp32, kind="ExternalInput").ap()
    bk = nc.dram_tensor("bk", [128, NH], fp32, kind="ExternalInput").ap()
    bv = nc.dram_tensor("bv", [1, HDc], fp32r, kind="ExternalInput").ap()
    bo = nc.dram_tensor("bo", [1, E], fp32r, kind="ExternalInput").ap()
    ones_d = nc.dram_tensor("ones", [128, 128], fp32r, kind="ExternalInput").ap()
    out = nc.dram_tensor("out", [NH * 128, E], fp32, kind="ExternalOutput").ap()

    r = lambda ap: ap  # operands already fp32r-typed

    with tile.TileContext(nc) as tc, ExitStack() as ctx:
        # ---- persistent SBUF ----
        persist = ctx.enter_context(tc.tile_pool(name="persist", bufs=1))
        qc = persist.tile([128, NH, S], fp32r)   # qT_c: [d, h, s]
        kc = persist.tile([128, NH, S], fp32r)   # kT_c: [d, h, s]
        vc = persist.tile([128, NT, HDc], fp32r)  # v_c:  [t_p, t_tile, h*128+d]
        consts = ctx.enter_context(tc.tile_pool(name="consts", bufs=1))
        ones_sb = consts.tile([128, 128], fp32r)
        nc.sync.dma_start(ones_sb[:], ones_d)
        ones_col = ones_sb[:, :1]
        ones_row = ones_sb[:1, :]
        bq_sb = consts.tile([128, NH], fp32)
        bk_sb = consts.tile([128, NH], fp32)
        bv_sb = consts.tile([1, HDc], fp32r)
        bo_sb = consts.tile([1, E], fp32r)
        nc.sync.dma_start(bq_sb[:], bq)
        nc.sync.dma_start(bk_sb[:], bk)
        nc.sync.dma_start(bv_sb[:], bv)
        nc.sync.dma_start(bo_sb[:], bo)

        # ================= Phase A: q/k/v projections =================
        with tc.tile_pool(name="wpool", bufs=2) as wpool, \
             tc.tile_pool(name="xin", bufs=6) as xin, \
             tc.tile_pool(name="pa_psum", bufs=2, space="PSUM") as pa:

            def proj_qk(w_dram, x_dram, bias_sb, dst):
                # dst[:, m, s] = sum_k w[k*128+p_m ...]: out (hd x S), +bias
                w_sb = wpool.tile([128, NK, HDc], fp32r, tag="w")
                for k in range(NK):
                    nc.sync.dma_start(
                        w_sb[:, k, :],
                        w_dram.rearrange("(k p) c -> k p c", p=128)[k])
                xt = x_dram.rearrange("(k p) s -> k p s", p=128)
                for s in range(NPC):
                    ps = [pa.tile([128, PCH], fp32, tag=f"pa{m}", name=f"pa{m}")
                          for m in range(NH)]
                    for k in range(NK):
                        xtile = xin.tile([128, PCH], fp32r, tag="xin")
                        nc.sync.dma_start(
                            xtile[:], xt[k, :, s * PCH:(s + 1) * PCH])
                        for m in range(NH):
                            nc.tensor.matmul(
                                ps[m][:], r(w_sb[:, k, m * 128:(m + 1) * 128]),
                                r(xtile[:]), start=(k == 0), stop=(k == NK - 1))
                    for m in range(NH):
                        nc.vector.tensor_scalar_add(
                            dst[:, m, s * PCH:(s + 1) * PCH], ps[m][:],
                            bias_sb[:, m:m + 1])

            def proj_v():
                # vc[:, tt, c] = sum_k xvT[k*128+p, ...]: v natural (S x hd), +bv
                w_sb = wpool.tile([128, NK, HDc], fp32r, tag="w")
                for k in range(NK):
                    nc.sync.dma_start(
                        w_sb[:, k, :],
                        wvT.rearrange("(k p) c -> k p c", p=128)[k])
                xt = xvT.rearrange("(k p) s -> k p s", p=128)
                for tt in range(NT):
                    ps = pa.tile([128, HDc], fp32, tag="pa0")
                    for k in range(NK):
                        xtile = xin.tile([128, 128], fp32r, tag="xvin")
                        nc.sync.dma_start(
                            xtile[:], xt[k, :, tt * 128:(tt + 1) * 128])
                        # lhsT = xvT tile (E x t), rhs = w_sb k-slice (E x hd)
                        nc.tensor.matmul(
                            ps[:], r(xtile[:]), r(w_sb[:, k, :]),
                            start=(k == 0), stop=False)
                    # bias: += ones_col(t) x bv(hd)
                    nc.tensor.matmul(ps[:], r(ones_row), r(bv_sb[:]),
                                     start=False, stop=True)
                    nc.vector.tensor_copy(vc[:, tt, :], ps[:])

            proj_qk(wqT, xqT, bq_sb, qc)
            proj_qk(wkT, xkT, bk_sb, kc)
            proj_v()

        # ================= Phase B: attention per head ================
        ocp = ctx.enter_context(tc.tile_pool(name="ocp", bufs=1))
        oc = ocp.tile([128, NH, S], fp32r)   # outT: [d, h, s]
        with tc.tile_pool(name="expp", bufs=2) as expp, \
             tc.tile_pool(name="bsc", bufs=2) as bsc, \
             tc.tile_pool(name="st_psum", bufs=4, space="PSUM") as stp, \
             tc.tile_pool(name="ot_psum", bufs=2, space="PSUM") as otp, \
             tc.tile_pool(name="dn_psum", bufs=2, space="PSUM") as dnp:

            def scores_exp(h, c):
                # scores^T tiles + exp -> expT chunk
                cs = slice(c * CH, (c + 1) * CH)
                expT = expp.tile([128, NT, CH], fp32r, tag="expT",
                                 name=f"expT_{h}_{c}")
                for tt in range(NT):
                    ps = stp.tile([128, CH], fp32, tag="st", name="st")
                    nc.tensor.matmul(
                        ps[:], r(kc[:, h, tt * 128:(tt + 1) * 128]),
                        r(qc[:, h, cs]), start=True, stop=True)
                    nc.scalar.activation(expT[:, tt, :], ps[:],
                                         AF.Exp, scale=inv_sqrt_dk)
                return expT

            def attn_tail(h, c, expT):
                # denom + outT + normalize for an exp'd chunk
                cs = slice(c * CH, (c + 1) * CH)
                acc = bsc.tile([128, CH], fp32r, tag="acc", name="acc")
                nc.vector.tensor_add(acc[:], expT[:, 0, :], expT[:, 1, :])
                for tt in range(2, NT):
                    nc.vector.tensor_add(acc[:], acc[:], expT[:, tt, :])
                dn = dnp.tile([1, CH], fp32, tag="dn", name="dn")
                nc.tensor.matmul(dn[:], r(ones_col), r(acc[:]),
                                 start=True, stop=True)
                dn_sb = bsc.tile([1, CH], fp32r, tag="dnsb", name="dnsb")
                nc.vector.tensor_copy(dn_sb[:], dn[:])
                dbc = dnp.tile([128, CH], fp32, tag="dn", name="dbc")
                nc.tensor.matmul(dbc[:], r(ones_row), r(dn_sb[:]),
                                 start=True, stop=True)
                rsc = bsc.tile([128, CH], fp32, tag="rsc", name="rsc")
                nc.vector.reciprocal(rsc[:], dbc[:])
                op = otp.tile([128, CH], fp32, tag="ot", name="ot")
                for tt in range(NT):
                    nc.tensor.matmul(
                        op[:], r(vc[:, tt, h * 128:(h + 1) * 128]),
                        r(expT[:, tt, :]), start=(tt == 0),
                        stop=(tt == NT - 1))
                nc.vector.tensor_mul(oc[:, h, cs], op[:], rsc[:])

            # 1-chunk software pipeline: scores/exp of i overlap tail of i-1
            pairs = [(h, c) for h in range(NH) for c in range(NCH)]
            prev = None
            for h, c in pairs:
                expT = scores_exp(h, c)
                if prev is not None:
                    attn_tail(*prev)
                prev = (h, c, expT)
            attn_tail(*prev)

        # ================= Phase C: output projection =================
        with tc.tile_pool(name="wo_in", bufs=6) as wo_in, \
             tc.tile_pool(name="osb", bufs=8) as osb, \
             tc.tile_pool(name="pc_psum", bufs=2, space="PSUM") as pc:
            wot = woT.rearrange("(k p) e -> k p e", p=128)
            for nn in range(NOC):
                ns = slice(nn * PCH, (nn + 1) * PCH)
                ps = [pc.tile([128, PCH], fp32, tag=f"pc{h}", name=f"pc{h}")
                      for h in range(NH)]
                for k in range(NK):
                    wtile = wo_in.tile([128, PCH], fp32r, tag="wo")
                    nc.sync.dma_start(wtile[:], wot[k, :, ns])
                    for h in range(NH):
                        # lhsT = X_h^T k-tile: strided view of outT
                        lhs = oc[:, h, :].rearrange("p (j i) -> p i j", i=NK)[:, k, :]
                        nc.tensor.matmul(ps[h][:], r(lhs), r(wtile[:]),
                                         start=(k == 0), stop=False)
                for h in range(NH):
                    nc.tensor.matmul(ps[h][:], r(ones_row),
                                     r(bo_sb[:, ns]), start=False, stop=True)
                    ot = osb.tile([128, PCH], fp32, tag="osb")
                    nc.vector.tensor_copy(ot[:], ps[h][:])
                    nc.sync.dma_start(out[h * 128:(h + 1) * 128, ns], ot[:])

    nc.compile()
    return nc


def shard_inputs(cfg: Cfg, query, key, value, Wq, bq, Wk, bk, Wv, bv, Wo, bo):
    """Build per-core in_maps from full inputs."""
    f = np.float32
    query, key, value = (np.asarray(a, f) for a in (query, key, value))
    Wq, Wk, Wv, Wo = (np.asarray(a, f) for a in (Wq, Wk, Wv, Wo))
    bq, bk, bv, bo = (np.asarray(a, f) for a in (bq, bk, bv, bo))
    NH, HDc = cfg.NH, cfg.HDc
    woT = np.ascontiguousarray(Wo.T)
    _ONES = np.ones((128, 128), np.float32)
    bo_r = np.ascontiguousarray(bo.reshape(1, -1))
    in_maps = []
    cores_per_batch = N_CORES // N_BATCH
    for c in range(N_CORES):
        n = c // cores_per_batch
        hs = (c % cores_per_batch) * HDc
        sl = slice(hs, hs + HDc)
        in_maps.append({
            "xqT": np.ascontiguousarray(query[n].T),
            "xkT": np.ascontiguousarray(key[n].T),
            "xvT": np.ascontiguousarray(value[n].T),
            "wqT": np.ascontiguousarray(Wq[sl].T),
            "wkT": np.ascontiguousarray(Wk[sl].T),
            "wvT": np.ascontiguousarray(Wv[sl].T),
            "woT": woT,
            "bq": np.ascontiguousarray(bq[sl].reshape(NH, 128).T),
            "bk": np.ascontiguousarray(bk[sl].reshape(NH, 128).T),
            "bv": np.ascontiguousarray(bv[sl].reshape(1, HDc)),
            "bo": bo_r,
            "ones": _ONES,
        })
    return in_maps


def gather_outputs(cfg: Cfg, results):
    """results: list of per-core {'out': (NH*128, E)} -> full (N, S, E)."""
    E = cfg.E
    full = np.empty((N_BATCH, SEQ, E), np.float32)
    cores_per_batch = N_CORES // N_BATCH
    rows = cfg.NH * 128
    for c in range(N_CORES):
        n = c // cores_per_batch
        r0 = (c % cores_per_batch) * rows
        full[n, r0:r0 + rows, :] = results[c]["out"]
    return full


_CACHE = {}


def kernel(**inputs) -> np.ndarray:
    from concourse.bass_utils import run_bass_kernel_spmd
    cfg = Cfg()
    if "nc" not in _CACHE:
        _CACHE["nc"] = build_program(cfg)
    nc = _CACHE["nc"]
    in_maps = shard_inputs(cfg, **inputs)
    res = run_bass_kernel_spmd(nc, in_maps, core_ids=list(range(N_CORES)))
    return gather_outputs(cfg, res.results)


# revision 13
# speedup vs baseline: 1.2499x; 1.2499x over previous
# Multi-head attention (N=2, S=2048, E=2048, H=16, Dk=128) on 8 NeuronCores.
#
# Sharding: 2 batches x 16 heads = 32 (n,h) pairs -> core c owns batch c//4,
# heads (c%4)*4 .. +4. The reference reshapes (N,H,S,Dk)->(N,S,H*Dk) without
# a head transpose, so rows [h*128,(h+1)*128) of the pre-projection matrix X
# (and hence of the final output) depend on head h only: each core computes
# 512 disjoint output rows and the host concatenates. No collectives.
#
# Device math per core (all matmuls fp32r, transposed layouts):
#   qT_c = Wq_c @ query[n].T   (hd x S)   kT_c same     v_c = value[n] @ Wv_c.T (S x hd)
#   sT   = k_h^T-tiles @ qT_h  (t x s)    expT = exp(sT/sqrt(Dk))
#   outT = v_h-tiles.T @ expT  (d x s)    denom: in-place add-tree + ones-matmul
#   out  = X_h @ Wo.T + bo  with X_h^T k-tiles = strided views of outT
#
# Perf notes (v2): all attention matmuls N=512 so the per-matmul fp32 weight
# load (~190ns) hides under the 225ns stream; DMA issue is spread over the
# sync/gpsimd/scalar queues (each dma_start costs ~0.6us of sequencer time);
# outT spills to DRAM between phases to stay under the SBUF cap.
import numpy as np

D_MODEL = 2048
NHEAD = 16
DK = 128
N_BATCH = 2
SEQ = 2048
N_CORES = 8
HEADS_PER_CORE = 4


class Cfg:
    def __init__(self, S=SEQ, E=D_MODEL, NH=HEADS_PER_CORE, CH=512):
        assert S % 128 == 0 and E % 128 == 0
        self.S = S          # sequence length
        self.E = E          # model dim (contraction for projections)
        self.NH = NH        # heads per core
        self.CH = CH        # s-chunk width for attention phase
        self.NK = E // 128  # contraction tiles for projections / O-proj
        self.NT = S // 128  # t tiles (attention contraction)
        self.HDc = NH * DK  # head dims per core
        self.RPH = (S * DK) // E  # output rows per head (=128 at full size)
        assert self.RPH == 128, "O-proj layout assumes 128 rows per head"
        self.NCH = S // CH  # number of s-chunks
        assert S % CH == 0 and CH >= 256  # fp32r full-rate needs N>=256
        self.PCH = 512      # projection / O-proj free-dim chunk
        self.NPC = S // self.PCH   # projection s-chunks
        self.NOC = E // self.PCH   # O-proj output chunks


def build_program(cfg: Cfg):
    import concourse.bass as bass
    import concourse.tile as tile
    from concourse import bacc, mybir
    from contextlib import ExitStack

    fp32 = mybir.dt.float32
    fp32r = mybir.dt.float32r
    AF = mybir.ActivationFunctionType

    S, E, NH, CH = cfg.S, cfg.E, cfg.NH, cfg.CH
    NK, NT, HDc = cfg.NK, cfg.NT, cfg.HDc
    PCH, NPC, NOC, NCH = cfg.PCH, cfg.NPC, cfg.NOC, cfg.NCH
    inv_sqrt_dk = 1.0 / float(np.sqrt(DK))

    nc = bacc.Bacc("TRN2", target_bir_lowering=False, debug=False,
                   num_devices=N_CORES)

    # DRAM I/O (per-core values supplied via in_maps)
    xqT = nc.dram_tensor("xqT", [E, S], fp32r, kind="ExternalInput").ap()
    xkT = nc.dram_tensor("xkT", [E, S], fp32r, kind="ExternalInput").ap()
    xvT = nc.dram_tensor("xvT", [E, S], fp32r, kind="ExternalInput").ap()
    wqT = nc.dram_tensor("wqT", [E, HDc], fp32r, kind="ExternalInput").ap()
    wkT = nc.dram_tensor("wkT", [E, HDc], fp32r, kind="ExternalInput").ap()
    wvT = nc.dram_tensor("wvT", [E, HDc], fp32r, kind="ExternalInput").ap()
    woT = nc.dram_tensor("woT", [E, E], fp32r, kind="ExternalInput").ap()
    bq = nc.dram_tensor("bq", [128, NH], fp32, kind="ExternalInput").ap()
    bk = nc.dram_tensor("bk", [128, NH], fp32, kind="ExternalInput").ap()
    bv = nc.dram_tensor("bv", [1, HDc], fp32r, kind="ExternalInput").ap()
    bo = nc.dram_tensor("bo", [1, E], fp32r, kind="ExternalInput").ap()
    ones_d = nc.dram_tensor("ones", [128, 128], fp32r, kind="ExternalInput").ap()
    out = nc.dram_tensor("out", [NH * 128, E], fp32, kind="ExternalOutput").ap()
    # outT spill buffer between attention and O-projection
    ocd = nc.dram_tensor("ocd", [NH, 128, S], fp32r).ap()

    with tile.TileContext(nc) as tc, ExitStack() as ctx:
        consts = ctx.enter_context(tc.tile_pool(name="consts", bufs=1))
        ones_sb = consts.tile([128, 128], fp32r)
        nc.sync.dma_start(ones_sb[:], ones_d)
        ones_col = ones_sb[:, :1]
        ones_row = ones_sb[:1, :]
        bq_sb = consts.tile([128, NH], fp32)
        bk_sb = consts.tile([128, NH], fp32)
        bv_sb = consts.tile([1, HDc], fp32r)
        bo_sb = consts.tile([1, E], fp32r)
        nc.sync.dma_start(bq_sb[:], bq)
        nc.sync.dma_start(bk_sb[:], bk)
        nc.sync.dma_start(bv_sb[:], bv)
        nc.sync.dma_start(bo_sb[:], bo)

        with tc.tile_pool(name="persist", bufs=1) as persist:
            qc = persist.tile([128, NH, S], fp32r)    # qT_c: [d, h, s]
            kc = persist.tile([128, NH, S], fp32r)    # kT_c: [d, h, s]
            vc = persist.tile([128, NT, HDc], fp32r)  # v_c: [t_p, t_t, h*128+d]

            # ============== Phase A: q/k/v projections ==============
            with tc.tile_pool(name="wpool", bufs=2) as wpool, \
                 tc.tile_pool(name="xin", bufs=6) as xin, \
                 tc.tile_pool(name="pa_psum", bufs=2, space="PSUM") as pa:

                def proj_qk(w_dram, x_dram, bias_sb, dst, eng):
                    # dst[:, m, s*] = W_c @ x^T  (hd x S), bias fused in evict
                    w_sb = wpool.tile([128, NK, HDc], fp32r, tag="w")
                    eng.dma_start(
                        w_sb[:], w_dram.rearrange("(k p) c -> p k c", p=128))
                    xt = x_dram.rearrange("(k p) s -> k p s", p=128)
                    for s in range(NPC):
                        ps = [pa.tile([128, PCH], fp32, tag=f"pa{m}",
                                      name=f"pa{m}") for m in range(NH)]
                        for k in range(NK):
                            xtile = xin.tile([128, PCH], fp32r, tag="xin")
                            eng.dma_start(
                                xtile[:], xt[k, :, s * PCH:(s + 1) * PCH])
                            for m in range(NH):
                                nc.tensor.matmul(
                                    ps[m][:],
                                    w_sb[:, k, m * 128:(m + 1) * 128],
                                    xtile[:], start=(k == 0),
                                    stop=(k == NK - 1))
                        for m in range(NH):
                            nc.vector.tensor_scalar_add(
                                dst[:, m, s * PCH:(s + 1) * PCH], ps[m][:],
                                bias_sb[:, m:m + 1])

                def proj_v(eng):
                    # vc natural (t x hd): stationary = xvT tiles, rhs = w
                    w_sb = wpool.tile([128, NK, HDc], fp32r, tag="w")
                    eng.dma_start(
                        w_sb[:], wvT.rearrange("(k p) c -> p k c", p=128))
                    xt = xvT.rearrange("(k p) s -> k p s", p=128)
                    for tc4 in range(NT // 4):   # groups of 4 t-tiles
                        ps = [pa.tile([128, HDc], fp32, tag=f"pa{j}",
                                      name=f"pav{j}") for j in range(4)]
                        for k in range(NK):
                            xtile = xin.tile([128, PCH], fp32r, tag="xin")
                            eng.dma_start(
                                xtile[:], xt[k, :, tc4 * 512:(tc4 + 1) * 512])
                            for j in range(4):
                                nc.tensor.matmul(
                                    ps[j][:], xtile[:, j * 128:(j + 1) * 128],
                                    w_sb[:, k, :], start=(k == 0), stop=False)
                        for j in range(4):
                            nc.tensor.matmul(ps[j][:], ones_row, bv_sb[:],
                                             start=False, stop=True)
                            nc.vector.tensor_copy(vc[:, tc4 * 4 + j, :],
                                                  ps[j][:])

                proj_qk(wqT, xqT, bq_sb, qc, nc.sync)
                proj_qk(wkT, xkT, bk_sb, kc, nc.gpsimd)
                proj_v(nc.scalar)

            # ============== Phase B: attention per head ==============
            with tc.tile_pool(name="expp", bufs=2) as expp, \
                 tc.tile_pool(name="bsc", bufs=2) as bsc, \
                 tc.tile_pool(name="ocsb", bufs=3) as ocsb, \
                 tc.tile_pool(name="st_psum", bufs=4, space="PSUM") as stp, \
                 tc.tile_pool(name="ot_psum", bufs=2, space="PSUM") as otp, \
                 tc.tile_pool(name="dn_psum", bufs=2, space="PSUM") as dnp:

                def scores_exp(h, c):
                    cs = slice(c * CH, (c + 1) * CH)
                    expT = expp.tile([128, NT, CH], fp32r, tag="expT",
                                     name=f"expT_{h}_{c}")
                    for tt in range(NT):
                        ps = stp.tile([128, CH], fp32, tag="st", name="st")
                        nc.tensor.matmul(
                            ps[:], kc[:, h, tt * 128:(tt + 1) * 128],
                            qc[:, h, cs], start=True, stop=True)
                        nc.scalar.activation(expT[:, tt, :], ps[:],
                                             AF.Exp, scale=inv_sqrt_dk)
                    return expT

                def attn_tail(h, c, expT):
                    cs = slice(c * CH, (c + 1) * CH)
                    # outT first (reads all of expT), then the in-place
                    # add-tree may clobber expT to build the denominator
                    op = otp.tile([128, CH], fp32, tag="ot", name="ot")
                    for tt in range(NT):
                        nc.tensor.matmul(
                            op[:], vc[:, tt, h * 128:(h + 1) * 128],
                            expT[:, tt, :], start=(tt == 0),
                            stop=(tt == NT - 1))
                    n = NT
                    while n > 2:
                        half = n // 2
                        nc.vector.tensor_add(expT[:, 0:half, :],
                                             expT[:, 0:half, :],
                                             expT[:, half:n, :])
                        n = half
                    acc = bsc.tile([128, CH], fp32r, tag="acc", name="acc")
                    nc.vector.tensor_add(acc[:], expT[:, 0, :], expT[:, 1, :])
                    dn = dnp.tile([1, CH], fp32, tag="dn", name="dn")
                    nc.tensor.matmul(dn[:], ones_col, acc[:],
                                     start=True, stop=True)
                    dn_sb = bsc.tile([1, CH], fp32r, tag="dnsb", name="dnsb")
                    nc.vector.tensor_copy(dn_sb[:], dn[:])
                    dbc = dnp.tile([128, CH], fp32, tag="dn", name="dbc")
                    nc.tensor.matmul(dbc[:], ones_row, dn_sb[:],
                                     start=True, stop=True)
                    rsc = bsc.tile([128, CH], fp32, tag="rsc", name="rsc")
                    nc.vector.reciprocal_approx_fast(rsc[:], dbc[:])
                    oc_t = ocsb.tile([128, CH], fp32r, tag="oct", name="oct")
                    nc.vector.tensor_mul(oc_t[:], op[:], rsc[:])
                    nc.gpsimd.dma_start(ocd[h, :, cs], oc_t[:])

                # 1-chunk software pipeline
                pairs = [(h, c) for h in range(NH) for c in range(NCH)]
                prev = None
                for h, c in pairs:
                    expT = scores_exp(h, c)
                    if prev is not None:
                        attn_tail(*prev)
                    prev = (h, c, expT)
                attn_tail(*prev)

        # ============== Phase C: output projection ==============
        with tc.tile_pool(name="ocin", bufs=1) as ocin, \
             tc.tile_pool(name="wo_in", bufs=6) as wo_in, \
             tc.tile_pool(name="osb", bufs=8) as osb, \
             tc.tile_pool(name="pc_psum", bufs=2, space="PSUM") as pc:
            oc_h = []
            for h in range(NH):
                t = ocin.tile([128, S], fp32r, name=f"ocin{h}")
                nc.scalar.dma_start(t[:], ocd[h])
                oc_h.append(t)
            wot = woT.rearrange("(k p) e -> k p e", p=128)
            for nn in range(NOC):
                ns = slice(nn * PCH, (nn + 1) * PCH)
                ps = [pc.tile([128, PCH], fp32, tag=f"pc{h}", name=f"pc{h}")
                      for h in range(NH)]
                for k in range(NK):
                    wtile = wo_in.tile([128, PCH], fp32r, tag="wo")
                    eng = nc.sync if k % 2 == 0 else nc.gpsimd
                    eng.dma_start(wtile[:], wot[k, :, ns])
                    for h in range(NH):
                        # lhsT = X_h^T k-tile: strided view of outT
                        lhs = oc_h[h].rearrange(
                            "p (j i) -> p i j", i=NK)[:, k, :]
                        nc.tensor.matmul(ps[h][:], lhs, wtile[:],
                                         start=(k == 0), stop=False)
                for h in range(NH):
                    nc.tensor.matmul(ps[h][:], ones_row, bo_sb[:, ns],
                                     start=False, stop=True)
                    ot = osb.tile([128, PCH], fp32, tag="osb")
                    nc.vector.tensor_copy(ot[:], ps[h][:])
                    nc.sync.dma_start(out[h * 128:(h + 1) * 128, ns], ot[:])

    nc.compile()
    return nc


def shard_inputs(cfg: Cfg, query, key, value, Wq, bq, Wk, bk, Wv, bv, Wo, bo):
    """Build per-core in_maps from full inputs."""
    f = np.float32
    query, key, value = (np.asarray(a, f) for a in (query, key, value))
    Wq, Wk, Wv, Wo = (np.asarray(a, f) for a in (Wq, Wk, Wv, Wo))
    bq, bk, bv, bo = (np.asarray(a, f) for a in (bq, bk, bv, bo))
    NH, HDc = cfg.NH, cfg.HDc
    woT = np.ascontiguousarray(Wo.T)
    _ONES = np.ones((128, 128), np.float32)
    bo_r = np.ascontiguousarray(bo.reshape(1, -1))
    in_maps = []
    cores_per_batch = N_CORES // N_BATCH
    for c in range(N_CORES):
        n = c // cores_per_batch
        hs = (c % cores_per_batch) * HDc
        sl = slice(hs, hs + HDc)
        in_maps.append({
            "xqT": np.ascontiguousarray(query[n].T),
            "xkT": np.ascontiguousarray(key[n].T),
            "xvT": np.ascontiguousarray(value[n].T),
            "wqT": np.ascontiguousarray(Wq[sl].T),
            "wkT": np.ascontiguousarray(Wk[sl].T),
            "wvT": np.ascontiguousarray(Wv[sl].T),
            "woT": woT,
            "bq": np.ascontiguousarray(bq[sl].reshape(NH, 128).T),
            "bk": np.ascontiguousarray(bk[sl].reshape(NH, 128).T),
            "bv": np.ascontiguousarray(bv[sl].reshape(1, HDc)),
            "bo": bo_r,
            "ones": _ONES,
        })
    return in_maps


def gather_outputs(cfg: Cfg, results):
    """results: list of per-core {'out': (NH*128, E)} -> full (N, S, E)."""
    E = cfg.E
    full = np.empty((N_BATCH, SEQ, E), np.float32)
    cores_per_batch = N_CORES // N_BATCH
    rows = cfg.NH * 128
    for c in range(N_CORES):
        n = c // cores_per_batch
        r0 = (c % cores_per_batch) * rows
        full[n, r0:r0 + rows, :] = results[c]["out"]
    return full


_CACHE = {}


def kernel(**inputs) -> np.ndarray:
    from concourse.bass_utils import run_bass_kernel_spmd
    cfg = Cfg()
    if "nc" not in _CACHE:
        _CACHE["nc"] = build_program(cfg)
    nc = _CACHE["nc"]
    in_maps = shard_inputs(cfg, **inputs)
    res = run_bass_kernel_spmd(nc, in_maps, core_ids=list(range(N_CORES)))
    return gather_outputs(cfg, res.results)
